# revision 1
# baseline (speedup 1.0000x reference)
"""Trainium2 Bass kernel for ConvMessageAggregator (fp16, DVE/ACT balanced).

Computes, for each node n (messages: [N, 16, 688] fp32):
  f1[i] = relu(w10*x[i] + w11*x[i+2] + b1)      i in 0..13   (dilated 2-tap conv)
  f2[i] = relu(w20*f1[i] + w21*f1[i+2] + b2)    i in 0..11
  out   = relu(sum_k mlp_w[k] * f2[6+k] + mlp_b)             -> [N, 688]

Only x rows 6..15 are consumed, so the host stages just those 10 rows, cast
to fp16 (rel err ~1e-3, far inside the 2e-2 gate), and upcasts the fp16
result back to fp32: device HBM traffic is 22 B/node/msg (28.2 MB in +
2.8 MB out per core, ~91us of DMA at ~345 GB/s).

DVE scalar_tensor_tensor has NO fast perf mode (1x always), so all DVE work
uses ops with 16-bit perf modes: tensor_scalar (4x mode, 0.27 ns/el) and
tensor_tensor (2x mode, 0.54 ns/el). The MLP weights fold into per-row
relu2 ACT ops (G[k] = |w_k|*f2[k], signs handled by add/sub merges) so the
6->1 tree is pure TT. Engine balance per tile: DVE ~19.9us (conv scale/add
+ tree), ACT ~19.6us (both relus + final), which overlaps the ~11us of DMA.

Per-core pipeline (2048 nodes = 8 tiles of 128 partitions x 2 node-blocks,
free layout [row, block, msg] => all op operands contiguous step-1 views;
first two tiles split per block to cut the fill ramp, last tile's back half
split to shorten the drain tail):
  DMA   x[128, 10, 2, 688] fp16                 (HWDGE on sync)
  DVE   v1  = x_ot * r1          (ts 4x)
  DVE   v1 += x_pv               (TT 2x, in place)
  ACT   g1  = Relu(p1*v1 + b1)   (in place)
  DVE   v2  = g1_ot * r2         (ts)
  DVE   v2 += g1_pv              (TT)
  ACT   G[k] = Relu(|w_k|p2*v2[k] + |w_k|b2)  k=0..5 (in place on v2 rows)
  DVE   <=5 TT add/sub pairwise sign merges -> T
  ACT   out = Relu(sign*T + mlp_b) -> fp16
  DMA   out tile -> DRAM                        (HWDGE on sync)
"""

import sys

for _p in ("/opt/trn_rl_repo",):
    if _p not in sys.path:
        sys.path.insert(0, _p)

import numpy as np

import concourse.bass as bass
import concourse.tile as tile
from concourse import mybir
from concourse.bass_utils import run_bass_kernel_spmd

N_FULL, L, MSG = 16384, 16, 688
N_CORES = 8
N_LOCAL = N_FULL // N_CORES  # 2048
P = 128
TW = 2                        # node blocks per tile
NTILES = N_LOCAL // (P * TW)  # 8
R0, NROWS = 6, 10

F16 = mybir.dt.float16
F32 = mybir.dt.float32
AF = mybir.ActivationFunctionType
OP = mybir.AluOpType

GP_TREE_OPS = 0     # how many tree merges to push onto GpSimd (tunable)
ACCUM_CONV1 = False  # conv1 add via SBUF->SBUF accumulate-DMA instead of DVE TT
ACCUM_CONV2 = False # same for conv2 add


def _split_multi_waits(nc):
    """TPB instructions encode at most ONE semaphore wait; this walrus build's
    codegen rejects instructions with more. Hoist extra waits into standalone
    EventSemaphore ops on the same (in-order) sequencer."""
    for func in nc.m.functions:
        for bb in func.blocks:
            insts = list(bb.instructions)
            if not any(
                i.sync_info is not None and len(i.sync_info.on_wait) > 1
                for i in insts
            ):
                continue
            new = []
            for inst in insts:
                si = inst.sync_info
                if si is not None and len(si.on_wait) > 1:
                    waits = list(si.on_wait)
                    for j, w in enumerate(waits[:-1]):
                        new.append(
                            mybir.InstEventSemaphore(
                                name=f"{inst.name}-hoistw{j}",
                                engine=inst.engine,
                                sync_info=mybir.SyncInfo(on_wait=[w], on_update=[]),
                            )
                        )
                    inst.sync_info = mybir.SyncInfo(
                        on_wait=[waits[-1]], on_update=list(si.on_update)
                    )
                new.append(inst)
            bb.instructions = new


def _conv_split(wa, wb):
    """Factor pre[i] = wa*in[i] + wb*in[i+2] as p*(in[pv] + r*in[ot]),
    |r| <= 1, p signed."""
    if abs(wa) >= abs(wb):
        return wa, (wb / wa if wa != 0.0 else 0.0), 0, 2
    return wb, wa / wb, 2, 0


def build_program(w10, w11, b1, w20, w21, b2, mlp_w, mlp_b):
    nc = bass.Bass(trn_type="TRN2", name="conv_msg_agg")
    x = nc.dram_tensor("x", [N_LOCAL, NROWS, MSG], F16, kind="ExternalInput")
    out = nc.dram_tensor("out", [N_LOCAL, MSG], F16, kind="ExternalOutput")

    p1, r1, pv1, ot1 = _conv_split(w10, w11)
    p2, r2, pv2, ot2 = _conv_split(w20, w21)
    nzk = [k for k in range(6) if mlp_w[k] != 0.0]

    with tile.TileContext(nc) as tc:
        with (
            tc.tile_pool(name="bias", bufs=1) as pool_b,
            tc.tile_pool(name="xin", bufs=3) as pool_x,
            tc.tile_pool(name="v1p", bufs=2) as pool_1,
            tc.tile_pool(name="v2p", bufs=3) as pool_2,
            tc.tile_pool(name="outp", bufs=3) as pool_o,
        ):
            # [P,1] SBUF bias vectors for the ACT ops
            b1c = pool_b.tile([P, 1], F32, tag="b1")
            nc.vector.memset(b1c[:], b1)
            gbias = {}
            for k in nzk:
                gbias[k] = pool_b.tile([P, 1], F32, tag=f"gb{k}", name=f"gb{k}")
                nc.vector.memset(gbias[k][:], abs(mlp_w[k]) * b2)
            mbc = pool_b.tile([P, 1], F32, tag="mb")
            nc.vector.memset(mbc[:], mlp_b)

            for it in range(NTILES):
                n0 = it * TW * P
                xt = pool_x.tile([P, NROWS, TW, MSG], F16, tag="x")
                # early tiles: per-block loads + per-block conv1 front so
                # compute starts after half a transfer (ramp cut); last
                # tile: per-block back half so the tail chain overlaps.
                front_split = it <= 1
                back_split = it == NTILES - 1
                blks = (
                    [(blk, blk + 1) for blk in range(TW)]
                    if front_split
                    else [(0, TW)]
                )
                bblks = (
                    [(blk, blk + 1) for blk in range(TW)]
                    if back_split
                    else [(0, TW)]
                )
                for lo, hi in blks:
                    nc.sync.dma_start(
                        out=xt[:, :, lo:hi, :],
                        in_=x[n0 + lo * P : n0 + hi * P].rearrange(
                            "(b p) r m -> p r b m", b=hi - lo
                        ),
                    )

                # conv1: v1 = r1*x_ot (ts), v1 += x_pv (TT), relu (ACT)
                v1 = pool_1.tile([P, 8, TW, MSG], F16, tag="v1")
                if p1 == 0.0:
                    nc.vector.memset(v1[:], max(b1, 0.0))
                else:
                    for lo, hi in blks:
                        nc.vector.tensor_scalar_mul(
                            v1[:, :, lo:hi, :], xt[:, ot1 : ot1 + 8, lo:hi, :], r1
                        )
                        nc.vector.tensor_tensor(
                            out=v1[:, :, lo:hi, :],
                            in0=v1[:, :, lo:hi, :],
                            in1=xt[:, pv1 : pv1 + 8, lo:hi, :],
                            op=OP.add,
                        )
                        nc.scalar.activation(
                            out=v1[:, :, lo:hi, :], in_=v1[:, :, lo:hi, :],
                            func=AF.Relu, bias=b1c[:], scale=p1,
                        )

                # conv2: v2 = r2*g1_ot (ts), v2 += g1_pv (TT)
                v2 = pool_2.tile([P, 6, TW, MSG], F16, tag="v2")
                ot = pool_o.tile([P, TW, MSG], F16, tag="o")
                for lo, hi in bblks:
                    if p2 == 0.0:
                        nc.vector.memset(v2[:, :, lo:hi, :], 0.0)
                        eff_p2 = 0.0
                    else:
                        nc.vector.tensor_scalar_mul(
                            v2[:, :, lo:hi, :], v1[:, ot2 : ot2 + 6, lo:hi, :], r2
                        )
                        nc.vector.tensor_tensor(
                            out=v2[:, :, lo:hi, :],
                            in0=v2[:, :, lo:hi, :],
                            in1=v1[:, pv2 : pv2 + 6, lo:hi, :],
                            op=OP.add,
                        )
                        eff_p2 = p2

                    # G[k] = |w_k| * Relu(p2*v2[k] + b2) (ACT, in place)
                    terms = []  # (sign, row_ap)
                    for k in nzk:
                        aw = abs(mlp_w[k])
                        nc.scalar.activation(
                            out=v2[:, k, lo:hi, :],
                            in_=v2[:, k, lo:hi, :],
                            func=AF.Relu,
                            bias=gbias[k][:],
                            scale=aw * eff_p2,
                        )
                        terms.append((1 if mlp_w[k] > 0 else -1, v2[:, k, lo:hi, :]))

                    # pairwise sign-merge tree (TT add/sub)
                    while len(terms) > 1:
                        pos = [t for t in terms if t[0] > 0]
                        neg = [t for t in terms if t[0] < 0]
                        if len(pos) >= 2:
                            (sa, aa), (sb, ab) = pos[0], pos[1]
                            op = OP.add
                        elif len(neg) >= 2:
                            (sa, aa), (sb, ab) = neg[0], neg[1]
                            op = OP.add
                        else:  # one of each
                            (sa, aa), (sb, ab) = pos[0], neg[0]
                            op = OP.subtract
                        nc.vector.tensor_tensor(out=aa, in0=aa, in1=ab, op=op)
                        terms = [
                            t for t in terms if t[1] is not aa and t[1] is not ab
                        ]
                        terms.append((sa, aa))

                    if not terms:
                        nc.vector.memset(ot[:, lo:hi, :], max(mlp_b, 0.0))
                    else:
                        # out = Relu(sign*T + mlp_b) on ACT
                        nc.scalar.activation(
                            out=ot[:, lo:hi, :], in_=terms[0][1], func=AF.Relu,
                            bias=mbc[:], scale=float(terms[0][0]),
                        )
                    nc.sync.dma_start(
                        out=out[n0 + lo * P : n0 + hi * P].rearrange(
                            "(b p) m -> p b m", b=hi - lo
                        ),
                        in_=ot[:, lo:hi, :],
                    )
    _split_multi_waits(nc)
    return nc


def run(inputs, trace=False, **spmd_kwargs):
    """Build + run on 8 cores. Returns (full_output, BassKernelResults)."""
    msgs = np.asarray(inputs["messages"])
    assert msgs.shape == (N_FULL, L, MSG), msgs.shape
    xs = np.ascontiguousarray(msgs[:, R0 : R0 + NROWS, :], dtype=np.float16)

    c1w = np.asarray(inputs["conv1_w"], dtype=np.float64)
    c2w = np.asarray(inputs["conv2_w"], dtype=np.float64)
    mlw = np.asarray(inputs["mlp_w"], dtype=np.float64)
    nc = build_program(
        float(c1w[0]),
        float(c1w[1]),
        float(np.asarray(inputs["conv1_b"], dtype=np.float64)),
        float(c2w[0]),
        float(c2w[1]),
        float(np.asarray(inputs["conv2_b"], dtype=np.float64)),
        [float(v) for v in mlw],
        float(np.asarray(inputs["mlp_b"], dtype=np.float64)),
    )

    in_maps = [
        {"x": xs[i * N_LOCAL : (i + 1) * N_LOCAL]} for i in range(N_CORES)
    ]
    res = run_bass_kernel_spmd(
        nc, in_maps, core_ids=list(range(N_CORES)), trace=trace, **spmd_kwargs
    )
    full = np.concatenate([r["out"] for r in res.results], axis=0).astype(
        np.float32
    )
    return full, res


def kernel(**inputs) -> np.ndarray:
    return run(inputs, trace=False)[0]

